# revision 40
# baseline (speedup 1.0000x reference)
"""Trainium2 Bass kernel: MultiHeadSelfAttention (B=2, S=2048, D=1024, H=16).

Self-contained. Accepts FULL inputs, returns FULL output.

Sharding (8 cores, SPMD, no collectives):
  core c -> batch b = c // 4, lane j = c % 4. Within a batch the 16 heads
  are sorted by valid_len (desc) and dealt round-robin to the 4 lanes, so
  slot i on every core holds a head from rank-quartet i. Each core computes
  q/k/v projections for its 4 heads, attention, and the row-parallel
  partial of the output projection (A @ Wo[:, heads].T, shape (S, D)).
  Host sums the 4 partials per batch (partials emitted in bf16).

The program is specialized to per-slot QUERY BUDGETS: budget[i] =
ceil(max valid_len in rank-quartet i / 256) * 256. Query blocks beyond a
slot's budget are entirely masked rows, whose attention output is exactly
uniform (= mean of V), so they are filled from a precomputed mean-V column
instead of being computed. One program serves all 8 cores; distinct
budget tuples compile separately and are cached.

Device-side schedule: a single global instruction stream of attention
"sites" (one 512- or 256-query chunk of one head; the tail chunk of a
slot is half-width when the budget is odd in 256 units), each 8 steps
(2 key tiles per step). Projections (q/k/v), the per-chunk output
projections, and the mean-V fills are chopped into FILL UNITS that are
interleaved between site steps by a static deadline schedule; output-
projection units are demand-driven (pulled into steps where the PE
would otherwise wait on the exp pipeline, once their last input norm
has flushed), so the Tensor engine stays dense end to end.

Device-side math notes:
  - All matmuls run in bf16 (fp32 PSUM accumulation). Score matmuls for
    two consecutive key-tiles run CONCURRENTLY on disjoint PE row-groups
    (the head's 64 k/q dims are duplicated into both partition halves).
  - The reference masks ENTIRE query rows j >= valid_len to -1e6 before
    softmax, making those rows' attention exactly uniform (1/S each). For
    masked rows inside a computed chunk we multiply q by a row mask
    (computed ON DEVICE from a tiny per-row valid_len input via iota +
    per-partition compare): masked query -> scores all 0 -> exp all 1 ->
    uniform attention.
  - No max-subtraction in softmax: scores/8 are bounded, so exp() cannot
    overflow in fp32 and softmax is scale-invariant anyway.
  - V tiles carry 64 replicated ones-columns (cols 64..127), so attn@V
    yields the softmax denominator REPLICATED on partitions 64..127 of
    the same PSUM bank (a matmul costs N cycles regardless of M, so the
    extra columns are free). reciprocal_approx_fast (18-bit, ~5x faster
    than exact; fed through SBUF because it mis-reads PSUM operands)
    runs on those 64 lanes and the normalization multiply follows
    directly -- no DRAM re-partition bounce, and the PSUM accumulator is
    released within ~1us.
  - A short burst of junk matmuls at t=0 keeps the PE HAM activity
    monitor busy so the array is at 2.4 GHz (not the cold 1.2 GHz) when
    the first projection arrives.
  - Inputs are pre-arranged on the HOST so every input DMA is contiguous
    within each SBUF partition, and ALL input DMAs ride the single sync
    queue in strict FIFO priority order (wk, x[0:512], wq, wv, x tail,
    wo): SDMA engines round-robin across queues at packet granularity,
    so a single queue is the only way to give the first projection's
    data true priority. Half-width sites place each key tile's score
    block at a PSUM bank boundary (matmul outputs cannot start mid-bank).
  - bq/bk/bv are zeros in this problem's setup_inputs. bv/bo are folded in
    EXACTLY on the host (rows of attn sum to 1, so attn@(v+bv) = attn@v+bv).
    If bq/bk were ever nonzero we fall back to a numpy reference path.
"""

import numpy as np

B, S, D = 2, 2048, 1024
H, DH = 16, 64
HPG = 4                 # heads per core
GW = HPG * DH           # 256
P = 128
N_CORES = 8
NCH = S // 512          # query chunks

_PROGS = {}             # budgets tuple -> compiled Bacc


def _to_bf16(a):
    import ml_dtypes
    return np.ascontiguousarray(np.asarray(a, dtype=np.float32)
                                .astype(ml_dtypes.bfloat16))


def _emit(tc, aps, budgets):
    """Emit the per-core program. budgets: 4 per-slot query budgets."""
    from contextlib import ExitStack

    import concourse.mybir as mybir

    nc = tc.nc
    f32 = mybir.dt.float32
    bf16 = mybir.dt.bfloat16
    EXP = mybir.ActivationFunctionType.Exp

    xC, wqT, wkT, wvT, woT, out = (
        aps["xC"], aps["wqT"], aps["wkT"], aps["wvT"], aps["woT"],
        aps["out"],
    )
    # budgets are 256-granular: nq[h] = number of 256-query blocks
    # computed for slot h; a chunk's width is 512 or 256 (half tail).
    nq = [b // 256 for b in budgets]
    nchunks = [(q + 1) // 2 for q in nq]      # 512-chunks (incl. half)

    def chunk_w(h, i4):
        return min(512, (nq[h] - 2 * i4) * 256)

    ctx = ExitStack()
    with ctx:
        sb = ctx.enter_context(tc.tile_pool(name="sb", bufs=1))
        # PSUM banks: scores 2x[128,1024] (4) + proj 2x[128,512] (2)
        #           + attn@V accumulators 2x[128,512] (2) = 8.
        ps_s = ctx.enter_context(tc.tile_pool(name="ps_s", bufs=2,
                                              space="PSUM"))
        ps_p = ctx.enter_context(tc.tile_pool(name="ps_p", bufs=2,
                                              space="PSUM"))
        psav = ctx.enter_context(tc.tile_pool(name="psav", bufs=2,
                                              space="PSUM"))
        rot = ctx.enter_context(tc.tile_pool(name="rot", bufs=8))
        ost = ctx.enter_context(tc.tile_pool(name="ost", bufs=3))
        sml = ctx.enter_context(tc.tile_pool(name="sml", bufs=4))
        rbp = ctx.enter_context(tc.tile_pool(name="rbp", bufs=3))
        qdp = ctx.enter_context(tc.tile_pool(name="qdp", bufs=4))
        xw = ctx.enter_context(tc.tile_pool(name="xw", bufs=1))

        # persistent intermediates
        wo_sb = xw.tile([P, 2, D], bf16, name="wo")
        q_sb = [sb.tile([P, S], bf16, name=f"q{p}") for p in range(2)]
        k_sb = [sb.tile([P, S], bf16, name=f"k{p}") for p in range(2)]
        # V tiles: per head 64 value dims + 64 replicated ones columns.
        v_sb = [sb.tile([P, HPG, 2 * DH], bf16, name=f"v{t}")
                for t in range(16)]
        a_sb = [sb.tile([P, S], bf16, name=f"a{c}") for c in range(2)]
        meanv = sb.tile([64, HPG], bf16, name="meanv")
        # per-head k with the head's 64 dims duplicated into both partition
        # halves: lets two key-tiles' score matmuls run CONCURRENTLY on
        # disjoint PE row-groups (tile_position packing)
        khd = [sb.tile([P, S], bf16, name=f"khd{h}") for h in range(HPG)]

        # ---- PE warm-up: junk matmuls keep the HAM activity window busy
        # from t=0 so real matmuls run at 2.4 GHz (cold PE is 1.2 GHz).
        junk = xw.tile([P, 512], bf16, name="junk")
        nc.gpsimd.memset(junk[:], 0.5)
        for _ in range(14):
            wt = ps_p.tile([P, 512], f32, name="ps_p")
            nc.tensor.matmul(wt[:], junk[:, 0:P], junk[:],
                             start=True, stop=True)

        # ---- input loads. ALL input DMAs ride the ONE sync queue in
        # strict priority order: the SDMA engines round-robin between
        # queues that have work at packet granularity, so transfers on
        # different queues SHARE bandwidth -- a single FIFO queue is the
        # only way to make "wk before the x tail" actually hold. A single
        # dma_start is already split across all 16 SDMA engines, so one
        # queue still moves data at the full ~358 GB/s.
        x_sb = xw.tile([P, 4, 8, 512], bf16, name="x")
        wq_sb = xw.tile([P, 8, GW], bf16, name="wq")
        wk_sb = xw.tile([P, 8, GW], bf16, name="wk")
        wv_sb = xw.tile([P, 8, GW], bf16, name="wv")
        mk_sb = xw.tile([P, 2, S], bf16, name="mk")
        vl_sb = xw.tile([P, 2], f32, name="vl")
        # warm the ACT exp table-set (~2.7us load) during the DMA phase
        # so the first real exp doesn't pay it on the critical path
        warm_in = sml.tile([1, 8], f32, name="warm_in")
        warm_out = sml.tile([1, 8], f32, name="warm_out")
        nc.gpsimd.memset(warm_in[:], 0.0)

        # vl is tiny but every FIFO hop costs a ~0.6-1us completion
        # receipt before the next transfer starts -> keep it off sync.
        nc.scalar.dma_start(vl_sb[:], aps["vl"])
        nc.sync.dma_start(wk_sb[:], wkT)
        nc.sync.dma_start(x_sb[:, 0], xC[:, 0])
        nc.sync.dma_start(wq_sb[:], wqT)
        nc.sync.dma_start(wv_sb[:], wvT)
        for c in range(1, 4):
            nc.sync.dma_start(x_sb[:, c], xC[:, c])
        nc.sync.dma_start(wo_sb[:], woT)
        nc.scalar.activation(warm_out[:], warm_in[:], EXP,
                             bias=0.0, scale=0.125)

        # row mask computed on-device (iota vs per-row valid_len) -- keeps
        # the 1MB mask tensor out of the input-DMA critical path.
        it_sb = xw.tile([P, S], f32, name="it")
        nc.gpsimd.iota(it_sb[:], pattern=[[1, S]], base=0,
                       channel_multiplier=0,
                       allow_small_or_imprecise_dtypes=True)
        for pr in range(2):
            nc.vector.tensor_scalar(mk_sb[:, pr, :], it_sb[:],
                                    vl_sb[:, pr:pr + 1], None,
                                    op0=mybir.AluOpType.is_lt)

        # ---- projection / fill emitters ---------------------------------
        def acc8(w_ap_fn, x_ap_fn, width=512):
            """8-term contraction accumulated in one PSUM bank."""
            pt = ps_p.tile([P, 512], f32, name="ps_p")[:, :width]
            for d in range(8):
                nc.tensor.matmul(pt, w_ap_fn(d), x_ap_fn(d),
                                 start=(d == 0), stop=(d == 7))
            return pt

        def emit_k_chunk(mt, n4):
            cs = slice(n4 * 512, (n4 + 1) * 512)
            pt = acc8(
                lambda d: wk_sb[:, d, mt * P:(mt + 1) * P],
                lambda d: x_sb[:, n4, d, :])
            nc.vector.tensor_copy(k_sb[mt][:, cs], pt)
            for rr in range(2):
                h = 2 * mt + rr
                src_ap = k_sb[mt][64 * rr:64 * rr + 64, cs]
                eng = nc.sync if (n4 + rr) % 2 else nc.gpsimd
                eng.dma_start(khd[h][0:64, cs], src_ap)
                eng.dma_start(khd[h][64:128, cs], src_ap)

        def emit_q_chunk(mt, n4):
            cs = slice(n4 * 512, (n4 + 1) * 512)
            pt = acc8(
                lambda d: wq_sb[:, d, mt * P:(mt + 1) * P],
                lambda d: x_sb[:, n4, d, :])
            # fold the row mask into q (masked query -> q = 0)
            nc.vector.tensor_mul(q_sb[mt][:, cs], pt, mk_sb[:, mt, cs])
            for rr in range(2):
                if n4 >= nchunks[2 * mt + rr]:
                    continue
                qd = qdp.tile([P, 512], bf16, name="qd")
                qd_tiles[(mt, n4, rr)] = qd
                src_ap = q_sb[mt][64 * rr:64 * rr + 64, cs]
                eng = nc.sync if (n4 + rr) % 2 else nc.gpsimd
                eng.dma_start(qd[0:64, :], src_ap)
                eng.dma_start(qd[64:128, :], src_ap)

        def emit_v_tile(t):
            pt = acc8(
                lambda d: x_sb[:, t // 4, d, (t % 4) * P:(t % 4 + 1) * P],
                lambda d: wv_sb[:, d, :], width=GW)
            nc.any.memset(v_sb[t][:], 1.0)   # ones block at [:, :, 64:128]
            nc.vector.tensor_copy(
                v_sb[t][:, :, 0:DH],
                pt.rearrange("p (h e) -> p h e", h=HPG),
            )

        def emit_meanv():
            # mean of V per head (= output of fully-masked query rows)
            pmv = ps_p.tile([P, 512], f32, name="ps_p")[:, :HPG]
            for h in range(HPG):
                for jt in range(16):
                    nc.tensor.matmul(
                        pmv[:, h:h + 1],
                        v_sb[jt][:, h, :],
                        v_sb[jt][:, h, DH:DH + 1],  # a ones column
                        start=(jt == 0), stop=(jt == 15),
                    )
            nc.vector.tensor_scalar_mul(meanv[:], pmv[:DH, :], 1.0 / S)

        def emit_fills():
            # fully-masked query blocks: attention output is mean-of-V
            for pair in range(2):
                for rr in range(2):
                    h = 2 * pair + rr
                    start = nq[h] * 256
                    if start < S:
                        nc.vector.tensor_copy(
                            a_sb[pair][64 * rr:64 * rr + 64, start:S],
                            meanv[:, h:h + 1].to_broadcast((64, S - start)),
                        )

        def emit_final_t(i4, t4):
            """Output projection for one 128-row tile of query chunk i4.

            Single-bank 2-matmul accumulation per 512-col half: the two
            ps_p bufs double-buffer the halves so the PSUM->SBUF copy of
            one half overlaps the matmuls of the next.
            """
            t = i4 * 4 + t4
            ot = ost.tile([P, D], bf16, name="ot")
            for n2 in range(2):
                pf = ps_p.tile([P, 512], f32, name="ps_p")
                ns = slice(n2 * 512, (n2 + 1) * 512)
                for c in range(2):
                    nc.tensor.matmul(pf[:], a_sb[c][:, t * P:(t + 1) * P],
                                     wo_sb[:, c, ns],
                                     start=(c == 0), stop=(c == 1))
                nc.vector.tensor_copy(ot[:, ns], pf[:])
            (nc.sync if t % 2 else nc.gpsimd).dma_start(
                out[t * P:(t + 1) * P, :], ot[:])

        class Site:
            """One (chunk, pair, head-row) attention block, pipelined."""

            def __init__(self, i4, pair, rr):
                self.i4, self.pair, self.rr = i4, pair, rr
                self.h = 2 * pair + rr
                self.w = chunk_w(self.h, i4)
                self.rows = slice(64 * rr, 64 * rr + 64)
                self.qs = slice(i4 * 512, i4 * 512 + self.w)
                self.pav = psav.tile([P, 512], f32, name="psav")[:, :self.w]
                self.pses = []

            def emit_scores(self, jtp, direct=False):
                w = self.w
                # full 2-bank tile even for half-width sites: each key
                # tile's block starts at a PSUM bank boundary (matmul
                # outputs cannot start mid-bank).
                pse = ps_s.tile([P, 1024], f32, name="ps_s")
                if direct:
                    # head of the kernel: skip the khd/qd duplication DMAs
                    # (they sit on the critical path before the first exp)
                    for jj in range(2):
                        jt = jtp * 2 + jj
                        nc.tensor.matmul(
                            pse[:, jj * 512:jj * 512 + w],
                            k_sb[self.pair][self.rows,
                                            jt * P:(jt + 1) * P],
                            q_sb[self.pair][self.rows, self.qs],
                            start=True, stop=True,
                        )
                    self.pses.append(pse)
                    return
                # the two key-tiles use disjoint PE row-groups (partitions
                # 0-63 / 64-127 of the duplicated khd/qd tiles) and
                # different PSUM banks, so they execute concurrently
                qd = qd_tiles[(self.pair, self.i4, self.rr)]
                for jj in range(2):
                    jt = jtp * 2 + jj
                    half = slice(64 * jj, 64 * jj + 64)
                    # scores^T = k @ q^T for head h
                    nc.tensor.matmul(
                        pse[:, jj * 512:jj * 512 + w],
                        khd[self.h][half, jt * P:(jt + 1) * P],
                        qd[half, 0:w],
                        start=True, stop=True,
                    )
                self.pses.append(pse)

            def emit_exp(self, jtp):
                w = self.w
                ex = rot.tile([P, 2 * w], bf16, name="ex")
                src = self.pses[jtp].rearrange("p (b k) -> p b k", b=2)
                nc.scalar.activation(
                    ex[:].rearrange("p (b k) -> p b k", b=2),
                    src[:, :, 0:w], EXP, bias=0.0, scale=0.125)
                self.pses[jtp] = None
                self.exs = getattr(self, "exs", {})
                self.exs[jtp] = ex

            def emit_av(self, jtp):
                w = self.w
                ex = self.exs.pop(jtp)
                for jj in range(2):
                    jt = jtp * 2 + jj
                    nc.tensor.matmul(
                        self.pav,
                        v_sb[jt][:, self.h, :],
                        ex[:, jj * w:(jj + 1) * w],
                        start=(jtp == 0 and jj == 0),
                        stop=(jtp == 7 and jj == 1),
                    )

            def emit_norm(self):
                # softmax denominator arrives REPLICATED on partitions
                # 64..127 of pav (the ones-columns of V): reciprocal on 64
                # lanes, multiply, done -- pav freed immediately.
                # reciprocal_approx_fast mis-reads PSUM operands, so hop
                # the denominators through SBUF first (still ~1.5us
                # cheaper per site than the exact reciprocal).
                w = self.w
                dn = rbp.tile([64, w], f32, name="dn")
                nc.vector.tensor_copy(dn[:], self.pav[64:128, :])
                rb = rbp.tile([64, w], f32, name="rb")
                nc.vector.reciprocal_approx_fast(rb[:], dn[:])
                nc.vector.tensor_mul(
                    a_sb[self.pair][self.rows, self.qs],
                    self.pav[0:DH, :], rb[:])

        # ---- site sequence ----------------------------------------------
        # pair 0 leads, pair 1 lags one chunk.
        site_seq = []
        for i4 in range(NCH + 1):
            if i4 < NCH:
                for rr in range(2):
                    if i4 < nchunks[rr]:
                        site_seq.append((i4, 0, rr))
            if 1 <= i4:
                for rr in range(2):
                    if i4 - 1 < nchunks[2 + rr]:
                        site_seq.append((i4 - 1, 1, rr))
        n_sites = len(site_seq)
        first_use = {}          # (pair, i4) -> first site pos
        last_chunk_pos = {}     # chunk -> last site pos
        for pos, (i4, pair, rr) in enumerate(site_seq):
            first_use.setdefault((pair, i4), pos)
            last_chunk_pos[i4] = pos

        # ---- fill units with deadlines ----------------------------------
        # unit: (deadline(pos, step), order, kind, args)
        units = []

        def add(dl, kind, *args):
            units.append((dl, len(units), kind, args))

        # k pair0 chunks 1.. during site 0 (chunk c needed by step 2c)
        for c in range(1, NCH):
            add((0, max(0, 2 * c - 2)), "k", 0, c)
        # v tiles 2.. during site 0 (tile t needed by av at step (t//2)+1)
        for t in range(2, 16):
            add((0, max(0, (t - 1) // 2)), "v", t)
        add((1, 0), "meanv")
        add((1, 1), "mvfill")
        # k pair1 + q(p1, 0) spread over positions before the first pair1
        # site; q chunks one site ahead of first use.
        p1first = min(first_use[(1, i4)] for i4 in range(nchunks[2])) \
            if nchunks[2] else n_sites
        p1units = [("k", 1, c) for c in range(NCH)]
        span = max(1, p1first - 1)
        for i, u in enumerate(p1units):
            posi = 1 + (i * span) // len(p1units)
            add((min(posi, p1first - 1), 2 + 2 * (i % 3)), *u)
        for (pair, i4), fp in sorted(first_use.items(), key=lambda kv: kv[1]):
            if i4 == 0 and pair == 0:
                continue        # prologue
            add((max(1, fp - 1), 2), "q", pair, i4)
        # final-t units gate on the LAST site whose computed columns
        # overlap that t-tile's 128 columns (per pair & head-row); tiles
        # covered only by mean-V fills gate on the fills instead. Finals
        # are DEMAND-driven: never emitted on a deadline, only pulled
        # into steps where the PE would otherwise stall (inventory dry),
        # once their gate has passed. Leftovers drain at the tail.
        final_gate = {}
        for t in range(16):
            i4 = t // 4
            gate = 1                    # mvfill deadline is (1, 1)
            for pos, (si4, pair, rr) in enumerate(site_seq):
                if si4 == i4 and chunk_w(2 * pair + rr, si4) > (t % 4) * P:
                    gate = max(gate, pos + 1)
            final_gate[(i4, t % 4)] = gate
            add((10 ** 6, t), "final", i4, t % 4)
        units.sort(key=lambda u: (u[0], u[1]))

        emitted_q = set()
        qd_tiles = {}
        uidx = [0]

        def emit_unit(kind, args):
            if kind == "k":
                emit_k_chunk(args[0], args[1])
            elif kind == "q":
                pair, i4 = args
                if (pair, i4) not in emitted_q:
                    emitted_q.add((pair, i4))
                    emit_q_chunk(pair, i4)
            elif kind == "v":
                emit_v_tile(args[0])
            elif kind == "meanv":
                emit_meanv()
            elif kind == "mvfill":
                emit_fills()
            elif kind == "final":
                emit_final_t(args[0], args[1])

        def do_fills(pos, step, pull=True, strict=False):
            # strict: called BEFORE flush_prev() -- a final gated on THIS
            # position's pending norm must not be pulled yet.
            did = 0
            while uidx[0] < len(units) and units[uidx[0]][0] <= (pos, step):
                _, _, kind, args = units[uidx[0]]
                uidx[0] += 1
                emit_unit(kind, args)
                did += 1
            if pull and did == 0 and uidx[0] < len(units):
                # pull one unit forward to keep the PE fed: any projection
                # unit, or a final whose gating norm has already flushed
                lim = pos - 1 if strict else pos
                for i in range(uidx[0], len(units)):
                    if (units[i][2] != "final"
                            or final_gate[units[i][3]] <= lim):
                        _, _, kind, args = units.pop(i)
                        emit_unit(kind, args)
                        break

        # ---- prologue: k0+q0 first (they gate the first scores/exp);
        # v0/v1 follow (only needed by the first attn@V one step later).
        emit_k_chunk(0, 0)
        emitted_q.add((0, 0))
        emit_q_chunk(0, 0)
        emit_v_tile(0)
        emit_v_tile(1)

        # ---- main stream -------------------------------------------------
        prev = None

        def flush_prev():
            nonlocal prev
            if prev is not None:
                prev.emit_av(7)
                prev.emit_norm()
                prev = None

        for pos, (i4, pair, rr) in enumerate(site_seq):
            site = Site(i4, pair, rr)
            for j in range(8):
                if j == 0:
                    # pull here too: the flush's av7 waits on exp7 (~1us
                    # of ACT); a pulled unit keeps the PE busy meanwhile.
                    # strict: the pending flush's finals aren't legal yet.
                    do_fills(pos, 0, strict=True)
                    flush_prev()
                site.emit_scores(j, direct=(pos == 0))
                site.emit_exp(j)
                if j >= 1:
                    site.emit_av(j - 1)
                do_fills(pos, j)
            prev = site
        flush_prev()
        while uidx[0] < len(units):             # tail: remaining finals
            _, _, kind, args = units[uidx[0]]
            uidx[0] += 1
            emit_unit(kind, args)


def build_program(budgets):
    """Build + schedule + compile the per-core program (cached per key)."""
    budgets = tuple(budgets)
    if budgets in _PROGS:
        return _PROGS[budgets]

    import concourse.mybir as mybir
    import concourse.tile as tile
    from concourse import bacc

    nc = bacc.Bacc("TRN2", target_bir_lowering=False, debug=False)
    bf16 = mybir.dt.bfloat16
    aps = {
        "xC": nc.dram_tensor("xC", [P, 4, 8, 512], bf16,
                             kind="ExternalInput").ap(),
        "wqT": nc.dram_tensor("wqT", [P, 8, GW], bf16,
                              kind="ExternalInput").ap(),
        "wkT": nc.dram_tensor("wkT", [P, 8, GW], bf16,
                              kind="ExternalInput").ap(),
        "wvT": nc.dram_tensor("wvT", [P, 8, GW], bf16,
                              kind="ExternalInput").ap(),
        "woT": nc.dram_tensor("woT", [P, 2, D], bf16,
                              kind="ExternalInput").ap(),
        "vl": nc.dram_tensor("vl", [P, 2], mybir.dt.float32,
                             kind="ExternalInput").ap(),
        "out": nc.dram_tensor("out", [S, D], bf16, kind="ExternalOutput").ap(),
    }
    with tile.TileContext(nc) as tc:
        _emit(tc, aps, budgets)
    nc.compile()
    _PROGS[budgets] = nc
    return nc


def plan(valid_lens):
    """Head->core assignment and the compile-time budget tuple.

    Returns (budgets, heads_per_core): heads_per_core[c] lists the 4
    global head indices (within core c's batch) in slot order.
    """
    valid = np.asarray(valid_lens).reshape(B, H)
    heads_per_core = [None] * N_CORES
    quart_max = [0] * HPG
    for b in range(B):
        order = np.argsort(-valid[b], kind="stable")
        for j in range(HPG):
            hs = [int(order[4 * i + j]) for i in range(HPG)]
            heads_per_core[b * HPG + j] = hs
        for i in range(HPG):
            quart_max[i] = max(quart_max[i],
                               int(valid[b, order[4 * i]]))
    budgets = tuple(min(-(-m // 256) * 256, S) for m in quart_max)
    return budgets, heads_per_core


def make_in_maps(X, Wq, Wk, Wv, Wo, valid_lens):
    """Host-side sharding: build the 8 per-core input maps.

    All arrays are pre-arranged so each device DMA is contiguous within
    every SBUF partition:
      xC   [128, 4, 8, 512]: xC[p,c,d,s] = X[b].T[d*128+p, c*512+s]
      w*T  [128, 8, 256]:    w[p,d,g]    = W[rows].T[d*128+p, g]
      woT  [128, 2, 1024]:   wo[p,c,n]   = Wo[:, rows].T[c*128+p, n]
      mask [128, 2, 2048]
    """
    import ml_dtypes
    X = np.asarray(X, dtype=np.float32)
    valid = np.asarray(valid_lens).reshape(B, H)
    budgets, heads_per_core = plan(valid_lens)
    in_maps = []
    xCs = [np.ascontiguousarray(
        _to_bf16(X[b].T).reshape(8, P, 4, 512).transpose(1, 2, 0, 3))
        for b in range(B)]
    Wq, Wk, Wv, Wo = (np.asarray(a, np.float32) for a in (Wq, Wk, Wv, Wo))
    for c in range(N_CORES):
        b = c // HPG
        hs = heads_per_core[c]
        rows = np.concatenate([np.arange(h * DH, (h + 1) * DH) for h in hs])
        vl = np.empty((P, 2), dtype=np.float32)
        for p in range(2):
            for rr in range(2):
                h = hs[2 * p + rr]
                vl[64 * rr:64 * rr + 64, p] = float(valid[b, h])

        def wtile(w):
            return np.ascontiguousarray(
                _to_bf16(w[rows, :].T).reshape(8, P, GW).transpose(1, 0, 2))

        in_maps.append({
            "xC": xCs[b],
            "wqT": wtile(Wq),
            "wkT": wtile(Wk),
            "wvT": wtile(Wv),
            "woT": np.ascontiguousarray(
                _to_bf16(Wo[:, rows].T).reshape(2, P, D).transpose(1, 0, 2)),
            "vl": vl,
        })
    return budgets, in_maps


def assemble(results, Wo, bv, bo):
    """Host-side unshard: sum row-parallel partials, fold bv/bo exactly."""
    out = np.zeros((B, S, D), dtype=np.float32)
    for c in range(N_CORES):
        b = c // HPG
        out[b] += np.asarray(results[c]["out"], dtype=np.float32)
    bias = (np.asarray(bv, np.float32) @ np.asarray(Wo, np.float32).T
            + np.asarray(bo, np.float32))
    out += bias[None, None, :]
    return out


def _numpy_fallback(X, Wq, bq, Wk, bk, Wv, bv, Wo, bo, valid_lens):
    X = np.asarray(X, np.float32)
    q = (X @ np.asarray(Wq, np.float32).T + np.asarray(bq, np.float32))
    k = (X @ np.asarray(Wk, np.float32).T + np.asarray(bk, np.float32))
    v = (X @ np.asarray(Wv, np.float32).T + np.asarray(bv, np.float32))

    def split(y):
        return (y.reshape(B, S, H, DH).transpose(0, 2, 1, 3)
                .reshape(B * H, S, DH))

    q, k, v = split(q), split(k), split(v)
    s = np.einsum("bqd,bkd->bqk", q, k) / np.sqrt(DH).astype(np.float32)
    rm = (np.arange(S)[None, :]
          < np.asarray(valid_lens).reshape(-1)[:, None])
    s = np.where(rm[:, :, None], s, -1e6)
    s = s - s.max(axis=-1, keepdims=True)
    e = np.exp(s)
    attn = e / e.sum(axis=-1, keepdims=True)
    o = np.einsum("bqk,bkd->bqd", attn, v)
    o = o.reshape(B, H, S, DH).transpose(0, 2, 1, 3).reshape(B, S, D)
    return o @ np.asarray(Wo, np.float32).T + np.asarray(bo, np.float32)


def run_cores(budgets, in_maps, trace=False, **kw):
    """Run the compiled program on cores 0-7."""
    from concourse.bass_utils import run_bass_kernel_spmd

    nc = build_program(budgets)
    return run_bass_kernel_spmd(nc, in_maps, list(range(N_CORES)),
                                trace=trace, **kw)


def kernel(X, Wq, bq, Wk, bk, Wv, bv, Wo, bo, valid_lens):
    if np.any(np.asarray(bq)) or np.any(np.asarray(bk)):
        # never the case for this problem's setup_inputs (zeros);
        # exact fallback kept for safety.
        return _numpy_fallback(X, Wq, bq, Wk, bk, Wv, bv, Wo, bo, valid_lens)
    budgets, in_maps = make_in_maps(X, Wq, Wk, Wv, Wo, valid_lens)
    res = run_cores(budgets, in_maps, trace=False)
    return assemble(res.results, Wo, bv, bo)
